# revision 1
# baseline (speedup 1.0000x reference)
"""Trainium2 Bass kernel for nn_CrossRPEAttentionMulti.

Sharding: 8 cores = batch(4) x head-group(2). Each core computes, for its
(b, g): kT = Wk_g @ x_b^T, V = x_b @ Wv_g^T, transposed attention
S^T = kT^T-slices @ qT with RPE bias added on the key axis, exp (no max
subtraction -- logits are bounded ~|3|), unnormalized out^T = V_aug^T @ P^T
with an appended ones-column producing the softmax denominators, per-head
normalization via reciprocal + partition-broadcast, and the output
projection y_partial = out_norm^T^T @ Wp_g^T. Host sums the two group
partials per batch and adds the bias.

All matmuls run in float32r (1 cyc/row on the PE for free-dim >= 256,
~1.5e-4 relative error). The RPE bias table is precomputed on the host
(tiny: <0.1% of FLOPs), expanded to the key-tile layout, and streamed as
bf16.
"""
import numpy as np
import ml_dtypes

import concourse.mybir as mybir
import concourse.tile as tile
from concourse import bacc
from concourse.bass_utils import run_bass_kernel_spmd

f32 = mybir.dt.float32
f32r = mybir.dt.float32r
bf16 = mybir.dt.bfloat16

# -- static problem configuration (matches the reference module) --
B, C, H, G = 4, 1024, 16, 24
P_SP = G * G            # 576 spatial patches / modality
LQ = P_SP + 1           # 577 queries
NKV = 3 * P_SP + 1      # 1729 keys/values
HD = C // H             # 64
HPC = 8                 # heads per core (16 heads / 2 groups)
NCORES = 8

NPAD = 1792             # keys padded to 14*128
NT = NPAD // 128        # 14 key tiles
QPAD = 768              # queries padded to 512+256 (both chunks >=256)
NBLOCKS = [(0, 512), (512, 1024), (1024, 1536), (1536, 1792)]
VSTRIDE = 66            # per-head V cols: 64 dims + ones col + pad (fp32r needs even M)


def _build_nc():
    nc = bacc.Bacc("TRN2", target_bir_lowering=False, debug=False)

    import os
    dbg = os.environ.get("KDEBUG") == "1"
    xT = nc.dram_tensor("xT", [C, NPAD], f32r, kind="ExternalInput")
    wkT = nc.dram_tensor("wkT", [C, 512], f32r, kind="ExternalInput")
    wvT = nc.dram_tensor("wvT", [C, 512], f32r, kind="ExternalInput")
    wpT = nc.dram_tensor("wpT", [512, C], f32r, kind="ExternalInput")
    qT = nc.dram_tensor("qT", [128, 4, QPAD], f32r, kind="ExternalInput")
    biasx = nc.dram_tensor("biasx", [HPC, NT, 128, LQ], bf16, kind="ExternalInput")
    y = nc.dram_tensor("y", [LQ, C], f32, kind="ExternalOutput")
    if dbg:
        d_kT = nc.dram_tensor("d_kT", [128, 4, NPAD], f32r, kind="ExternalOutput")
        d_v = nc.dram_tensor("d_v", [128, NT, HPC * VSTRIDE], f32r, kind="ExternalOutput")
        d_rec = nc.dram_tensor("d_rec", [128, 4, LQ + 1], f32, kind="ExternalOutput")
        d_outT = nc.dram_tensor("d_outT", [128, 4, LQ + 1], f32r, kind="ExternalOutput")

    xTr = xT.rearrange("(j p) n -> j p n", p=128)

    with tile.TileContext(nc) as tc:
        with (
            tc.tile_pool(name="main", bufs=1) as main,
            tc.tile_pool(name="ptp", bufs=3) as ptp,
            tc.tile_pool(name="biasp", bufs=4) as biasp,
            tc.tile_pool(name="recp", bufs=2) as recp,
            tc.tile_pool(name="yp", bufs=2) as yp,
        ):
            kT_sb = main.tile([128, 4, NPAD], f32r)
            v_sb = main.tile([128, NT, HPC * VSTRIDE], f32r)
            qT_sb = main.tile([128, 4, QPAD], f32r)
            wpT_sb = main.tile([128, 4, C], f32r)
            outT = main.tile([128, 4, LQ + 1], f32r)
            rec_full = main.tile([128, 4, LQ + 1], f32)
            nc.vector.memset(rec_full[:, :, LQ:LQ + 1], 0.0)
            nc.vector.tensor_copy(outT[:, :, LQ:LQ + 1], rec_full[:, :, LQ:LQ + 1])

            nc.sync.dma_start(qT_sb, qT.ap())
            nc.sync.dma_start(wpT_sb, wpT.rearrange("(j p) n -> p j n", p=128))

            # ones column of V_aug (gives softmax denominators for free);
            # t=13 rows 65.. are x-padding -> keep their ones at 0.
            vre = v_sb.rearrange("p t (h e) -> p t h e", e=VSTRIDE)
            ones_f = main.tile([128, NT, HPC, 2], f32)
            nc.vector.memset(ones_f[:, :, :, 1], 0.0)
            nc.vector.memset(ones_f[:, 0:13, :, 0], 1.0)
            nc.vector.memset(ones_f[64:128, 13, :, 0], 0.0)
            nc.vector.memset(ones_f[64:65, 13, :, 0], 1.0)
            nc.vector.memset(ones_f[0:64, 13, :, 0], 1.0)
            nc.vector.tensor_copy(vre[:, :, :, 64:66], ones_f)

            # ---- phase 1+2: kT and V, streaming x^T blocks ----
            with (
                tc.tile_pool(name="wk", bufs=1) as wk,
                tc.tile_pool(name="xs", bufs=2) as xs,
                tc.tile_pool(name="psmm", bufs=3, space="PSUM") as psmm,
            ):
                wkT_sb = wk.tile([128, 8, 512], f32r)
                wvT_sb = wk.tile([128, 8, 512], f32r)
                nc.sync.dma_start(wkT_sb, wkT.rearrange("(j p) m -> p j m", p=128))
                nc.sync.dma_start(wvT_sb, wvT.rearrange("(j p) m -> p j m", p=128))

                for bi, (n0, n1) in enumerate(NBLOCKS):
                    w = n1 - n0
                    xblk = xs.tile([128, 8, 512], f32r, tag="xblk")
                    for kj in range(8):
                        nc.sync.dma_start(xblk[:, kj, 0:w], xTr[kj][:, n0:n1])
                    # kT rows for this n-block (all 4 c'-tiles)
                    for mt in range(4):
                        ps = psmm.tile([128, 512], f32, tag="ps")
                        for kj in range(8):
                            nc.tensor.matmul(
                                ps[:, 0:w],
                                wkT_sb[:, kj, mt * 128:(mt + 1) * 128],
                                xblk[:, kj, 0:w],
                                start=(kj == 0), stop=(kj == 7),
                            )
                        nc.vector.tensor_copy(kT_sb[:, mt, n0:n1], ps[:, 0:w])
                    # V tiles inside this n-block
                    for t in range(bi * 4, min(bi * 4 + 4, NT)):
                        rel = t * 128 - n0
                        ps = psmm.tile([128, 512], f32, tag="ps")
                        for kj in range(8):
                            nc.tensor.matmul(
                                ps,
                                xblk[:, kj, rel:rel + 128],
                                wvT_sb[:, kj, :],
                                start=(kj == 0), stop=(kj == 7),
                            )
                        nc.scalar.copy(
                            vre[:, t, :, 0:64],
                            ps.rearrange("p (h e) -> p h e", e=64),
                        )

            # ---- phase 3: attention ----
            with (
                tc.tile_pool(name="psst", bufs=2, space="PSUM") as psst,
                tc.tile_pool(name="psout", bufs=2, space="PSUM") as psout,
            ):
                for h in range(HPC):
                    pb = (h % 2) * 64
                    j = h // 2
                    ops = psout.tile([66, LQ + 1], f32, tag="ops")
                    for t in range(NT):
                        bt = biasp.tile([128, LQ], bf16, tag="bt")
                        nc.sync.dma_start(bt, biasx.ap()[h, t])
                        st = psst.tile([128, QPAD], f32, tag="st")
                        lk = kT_sb[pb:pb + 64, j, t * 128:(t + 1) * 128]
                        for (q0, q1) in ((0, 512), (512, QPAD)):
                            nc.tensor.matmul(
                                st[:, q0:q1], lk, qT_sb[pb:pb + 64, j, q0:q1],
                                start=True, stop=True,
                            )
                        nc.vector.tensor_add(
                            out=st[:, 0:LQ], in0=st[:, 0:LQ], in1=bt)
                        pt = ptp.tile([128, LQ + 1], f32r, tag="pt")
                        nc.scalar.activation(
                            pt, st[:, 0:LQ + 1], mybir.ActivationFunctionType.Exp)
                        lv = v_sb[:, t, h * VSTRIDE:(h + 1) * VSTRIDE]
                        for (q0, q1) in ((0, 512), (512, LQ + 1)):
                            nc.tensor.matmul(
                                ops[:, q0:q1], lv, pt[:, q0:q1],
                                start=(t == 0), stop=(t == NT - 1),
                            )
                    rec = recp.tile([1, LQ], f32, tag="rec")
                    nc.vector.reciprocal(rec, ops[64:65, 0:LQ])
                    rbc = recp.tile([64, LQ], f32, tag="rbc")
                    nc.gpsimd.partition_broadcast(rbc, rec)
                    nc.vector.tensor_copy(rec_full[pb:pb + 64, j, 0:LQ], rbc)
                    nc.vector.tensor_copy(outT[pb:pb + 64, j, 0:LQ], ops[0:64, 0:LQ])

            # ---- phase 4: normalize + projection ----
            with tc.tile_pool(name="pspj", bufs=2, space="PSUM") as pspj:
                if dbg:
                    nc.sync.dma_start(d_kT.ap(), kT_sb)
                    nc.sync.dma_start(d_v.ap(), v_sb)
                    nc.sync.dma_start(d_rec.ap(), rec_full)
                    nc.sync.dma_start(d_outT.ap(), outT)
                nc.vector.tensor_mul(out=outT, in0=outT, in1=rec_full)
                for mt in range(5):
                    m0 = mt * 128
                    mcols = 66 if mt == 4 else 128   # lhsT free width (even)
                    mrows = 65 if mt == 4 else 128   # valid output rows
                    ps = pspj.tile([128, C], f32, tag="pp")
                    for j in range(4):
                        for (c0, c1) in ((0, 512), (512, C)):
                            nc.tensor.matmul(
                                ps[:mcols, c0:c1],
                                outT[:, j, m0:m0 + mcols],
                                wpT_sb[:, j, c0:c1],
                                start=(j == 0), stop=(j == 3),
                            )
                    for (c0, c1) in ((0, 512), (512, C)):
                        yt = yp.tile([128, 512], f32, tag="yt")
                        nc.vector.tensor_copy(yt[:mrows], ps[:mrows, c0:c1])
                        nc.sync.dma_start(y.ap()[m0:m0 + mrows, c0:c1], yt[:mrows])

    nc.finalize()
    return nc


_NC_CACHE = None


def _get_nc():
    global _NC_CACHE
    if _NC_CACHE is None:
        _NC_CACHE = _build_nc()
    return _NC_CACHE


def _host_prep(x, q_learned, pos_embed, Wk, Wv, Wp, rpe_W, rp_bucket):
    """Build the 8 per-core input maps."""
    x = np.asarray(x, dtype=np.float32)
    q_ = (np.asarray(q_learned, np.float32) + np.asarray(pos_embed, np.float32))[0]
    Wk = np.asarray(Wk, np.float32)
    Wv = np.asarray(Wv, np.float32)
    Wp = np.asarray(Wp, np.float32)
    rpe_W = np.asarray(rpe_W, np.float32)
    rp_bucket = np.asarray(rp_bucket)

    scale = HD ** -0.5

    # RPE bias, expanded to key-tile layout, transposed: biasx[h, t, p, q]
    qh = q_.reshape(LQ, H, HD)
    rpe_tab = np.einsum('qhd,dn->hqn', qh, rpe_W)                  # (H, LQ, nb)
    rpe = np.take_along_axis(
        rpe_tab, np.broadcast_to(rp_bucket[None], (H, LQ, LQ)), axis=-1
    )                                                              # (H, q, j')
    n_idx = np.arange(NPAD)
    jcol = np.where(n_idx == 0, 0, 1 + (n_idx - 1) % P_SP)         # (NPAD,)
    biasx = rpe[:, :, jcol]                                        # (H, q, n)
    biasx[:, :, NKV:] = 0.0
    biasx = np.ascontiguousarray(
        biasx.transpose(0, 2, 1)                                   # (H, n, q)
    ).reshape(H, NT, 128, LQ).astype(ml_dtypes.bfloat16)

    # qT per group, scaled, padded: (2, 128, 4, QPAD)
    qTg = np.zeros((2, 512, QPAD), np.float32)
    qTg[:, :, :LQ] = (q_.T * scale).reshape(2, 512, LQ)
    qTg = qTg.reshape(2, 4, 128, QPAD).transpose(0, 2, 1, 3).copy()

    per_group = []
    for g in range(2):
        sl = slice(g * 512, (g + 1) * 512)
        per_group.append({
            "wkT": np.ascontiguousarray(Wk[sl, :].T),
            "wvT": np.ascontiguousarray(Wv[sl, :].T),
            "wpT": np.ascontiguousarray(Wp[:, sl].T),
            "qT": np.ascontiguousarray(qTg[g]),
            "biasx": np.ascontiguousarray(biasx[g * HPC:(g + 1) * HPC]),
        })

    in_maps = []
    for b in range(B):
        xTb = np.zeros((C, NPAD), np.float32)
        xTb[:, :NKV] = x[b].T
        for g in range(2):
            m = dict(per_group[g])
            m["xT"] = xTb
            in_maps.append(m)
    return in_maps


def kernel(x, q_learned, pos_embed, Wk, Wv, Wp, bp, rpe_W, rp_bucket):
    in_maps = _host_prep(x, q_learned, pos_embed, Wk, Wv, Wp, rpe_W, rp_bucket)
    nc = _get_nc()

    last_err = None
    for _attempt in range(3):
        try:
            res = run_bass_kernel_spmd(nc, in_maps, core_ids=list(range(NCORES)))
            break
        except Exception as e:  # wedged-device recovery: retry
            last_err = e
    else:
        raise last_err

    bp = np.asarray(bp, np.float32)
    out = np.empty((B, LQ, C), np.float32)
    for b in range(B):
        out[b] = res.results[2 * b]["y"] + res.results[2 * b + 1]["y"] + bp
    return out



# revision 98
# speedup vs baseline: 31053.3682x; 31053.3682x over previous
"""Trainium2 Bass kernel for nn_CrossRPEAttentionMulti.

Sharding: 8 cores = batch(4) x head-group(2). Each core computes, for its
(b, g): kT = Wk_g @ x_b^T, V = x_b @ Wv_g^T, transposed attention
S^T = kT-slices @ qT, exp (no max subtraction -- logits are bounded ~|6|)
producing bf16 probabilities, unnormalized out^T = V_aug^T @ P^T with an
appended ones-column producing the softmax denominators, per-head
normalization fused into the PSUM->SBUF copy, and the output projection
y_partial = out_norm^T^T @ Wp_g^T. Host sums the two group partials per
batch and adds the output bias.

RPE bias handling: the bucketed-distance bias decomposes into
  bias[h, q, key] = ones[key] * A[h, q] + is_cls[key] * Cc[h, q] + LOCAL
where LOCAL is nonzero only for key/query pairs within 3 grid rows
(round(dist) <= 3). Keys are reordered (cls last) so each 128-key tile's
LOCAL support is 1-2 contiguous query windows. The rank-1 terms ride as
two extra contraction rows (64: ones, 65: cls indicator) inside the S^T
matmul (free - the PE charges only for moving columns), and LOCAL is
accumulated into the S^T PSUM group by bf16 identity-matmuls over the
narrow windows only. k/q/bias/P/V run in bf16 (measured end-to-end error
~2.6e-3 vs the 2e-2 gate); the dense kv GEMMs and projection stay f32r.
"""
import numpy as np
import ml_dtypes

import concourse.mybir as mybir
import concourse.tile as tile
from concourse import bacc
from concourse.bass_utils import run_bass_kernel_spmd

f32 = mybir.dt.float32
f32r = mybir.dt.float32r
bf16 = mybir.dt.bfloat16

# -- static problem configuration (matches the reference module) --
B, C, H, G = 4, 1024, 16, 24
P_SP = G * G            # 576 spatial patches / modality
LQ = P_SP + 1           # 577 queries
NKV = 3 * P_SP + 1      # 1729 keys/values
HD = C // H             # 64
HPC = 8                 # heads per core (16 heads / 2 groups)
NCORES = 8

# key order on device: 3*576 spatial keys (modality-major), cls at 1728
NPAD = 1792             # keys padded to 14*128
NT = NPAD // 128        # 14 key tiles
CLS_KEY = 3 * P_SP      # 1728
QPAD = 580              # queries padded: psum banks split at 512
QT0, QT1 = 512, QPAD
VSTRIDE = 66            # per-head V cols: 64 dims + ones col + pad


def _windows():
    """Per-key-tile query windows where the non-rank-1 RPE residual lives.

    A tile's spatial keys span grid rows [rlo, rhi]; LOCAL is nonzero for
    queries within +-3 grid rows (round(dist) <= 3 => bucket <= 2). Tiles
    crossing a modality boundary get two windows.
    """
    wins = []
    for t in range(NT):
        lo, hi = t * 128, min(t * 128 + 128, CLS_KEY) - 1
        parts = []
        spans = []
        plo, phi = lo % P_SP, hi % P_SP
        if plo <= phi:
            spans.append((plo, phi))
        else:  # wraps a modality boundary
            spans.append((plo, P_SP - 1))
            spans.append((0, phi))
        for (a, b) in spans:
            r0 = max(0, a // G - 3)
            r1 = min(G - 1, b // G + 3)
            parts.append((1 + r0 * G, 1 + (r1 + 1) * G))
        wins.append(parts)
    return wins


WINDOWS = _windows()
WTOT = sum(q1 - q0 for parts in WINDOWS for (q0, q1) in parts)


def _build_nc():
    nc = bacc.Bacc("TRN2", target_bir_lowering=False, debug=False)

    xT = nc.dram_tensor("xT", [8, 128, NPAD], bf16, kind="ExternalInput")
    wkT = nc.dram_tensor("wkT", [C, 512], bf16, kind="ExternalInput")
    wvT = nc.dram_tensor("wvT", [C, 512], bf16, kind="ExternalInput")
    wpT = nc.dram_tensor("wpT", [512, C], f32r, kind="ExternalInput")
    qT = nc.dram_tensor("qT", [128, HPC, QPAD], bf16, kind="ExternalInput")
    kaux = nc.dram_tensor("kaux", [2, HPC, 128], bf16, kind="ExternalInput")
    identb = nc.dram_tensor("identb", [128, 128], bf16, kind="ExternalInput")
    biasw = nc.dram_tensor("biasw", [HPC, 128, WTOT], bf16, kind="ExternalInput")
    y = nc.dram_tensor("y", [LQ, C], bf16, kind="ExternalOutput")

    xTr = xT.rearrange("j p n -> p j n")

    with tile.TileContext(nc) as tc:
        with (
            tc.tile_pool(name="main", bufs=1) as main,
            tc.tile_pool(name="ptp", bufs=6) as ptp,
            tc.tile_pool(name="ptp2", bufs=7) as ptp2,
            tc.tile_pool(name="biasp", bufs=6) as biasp,
            tc.tile_pool(name="recp", bufs=2) as recp,
            tc.tile_pool(name="yp", bufs=2) as yp,
        ):
            # per-head kT slots: rows 0:64 head channels, 64 ones (valid
            # keys), 65 cls indicator -- the S matmul contracts over 0:66
            kT_sb = main.tile([128, HPC, NPAD], bf16)
            v_sb = main.tile([128, NT, HPC * VSTRIDE], bf16)
            qT_sb = main.tile([128, HPC, QPAD], bf16)
            ident_sb = main.tile([128, 128], bf16)
            outT = main.tile([128, 4, QPAD], f32r)
            zsrc = main.tile([128, QPAD], f32)
            # pad query columns [LQ:QPAD] must stay zero through the
            # projection; real columns are overwritten per head later.
            nc.vector.memset(zsrc, 0.0)
            for j in range(4):
                nc.vector.tensor_copy(outT[:, j, :], zsrc)



            # ones column of V_aug (gives softmax denominators for free);
            # t=13 rows 65.. are x-padding (row 64 = cls) -> ones at 0.
            vre = v_sb.rearrange("p t (h e) -> p t h e", e=VSTRIDE)
            ones_f = main.tile([128, NT, HPC, 2], bf16)
            nc.vector.memset(ones_f[:, :, :, 1], 0.0)
            nc.vector.memset(ones_f[:, 0:13, :, 0], 1.0)
            nc.vector.memset(ones_f[64:128, 13, :, 0], 0.0)
            nc.vector.memset(ones_f[64:65, 13, :, 0], 1.0)
            nc.vector.memset(ones_f[0:64, 13, :, 0], 1.0)
            nc.vector.tensor_copy(vre[:, :, :, 64:66], ones_f)

            # window -> (bank parts, biasw column offsets), static per tile
            woff = []
            off = 0
            for parts in WINDOWS:
                lst = []
                for (q0, q1) in parts:
                    lst.append((q0, q1, off))
                    off += q1 - q0
                woff.append(lst)

            def emit_tile_at(h, t, st2, base, bh):
                """S^T chunks + LOCAL bias idents for tile t at column
                offset `base` of PSUM tile st2, respecting 512-col (2KB)
                bank boundaries for matmul outputs and accumulation groups.
                """
                kd = 66 if t == NT - 1 else 65  # cls row only in tile 13
                lk = kT_sb[0:kd, h, t * 128:(t + 1) * 128]
                lo, hi = base, base + QPAD
                # bank segments of [lo, hi)
                segs = []
                a = lo
                while a < hi:
                    b = min(hi, (a // 512 + 1) * 512)
                    segs.append((a, b))
                    a = b
                # ident pieces per bank segment
                pieces = {s: [] for s in segs}
                for (q0, q1, o) in woff[t]:
                    a0, a1 = base + q0, base + q1
                    for (sa, sb) in segs:
                        pa, pb = max(a0, sa), min(a1, sb)
                        if pa < pb:
                            pieces[(sa, sb)].append(
                                (pa, pb, o + (pa - a0)))
                for (sa, sb) in segs:
                    ps_ = pieces[(sa, sb)]
                    nc.tensor.matmul(
                        st2[:, sa:sb], lk, qT_sb[0:kd, h, sa - base:sb - base],
                        start=True, stop=not ps_,
                    )
                    for i, (pa, pb, o) in enumerate(ps_):
                        nc.tensor.matmul(
                            st2[:, pa:pb], ident_sb, bh[:, o:o + pb - pa],
                            start=False, stop=(i == len(ps_) - 1),
                        )

            def emit_pv_at(h, t, pt2, base, ops):
                lv = v_sb[:, t, h * VSTRIDE:(h + 1) * VSTRIDE]
                for (q0, q1) in ((0, QT0), (QT0, QT1)):
                    nc.tensor.matmul(
                        ops[:, q0:q1], lv, pt2[:, base + q0:base + q1],
                        start=(t == 0), stop=(t == NT - 1),
                    )

            def emit_sexp(h, t, pool, bh):
                """S^T chunks + LOCAL bias idents + exp for one key tile."""
                st = pool.tile([128, QPAD], f32, tag="st")
                emit_tile_at(h, t, st, 0, bh)
                pt = ptp.tile([128, QPAD], bf16, tag="pt")
                nc.scalar.activation(pt, st, mybir.ActivationFunctionType.Exp)
                return pt

            def emit_pv(h, t, pt, ops):
                emit_pv_at(h, t, pt, 0, ops)

            def emit_epilogue(h, ops):
                # denominators + fused normalize-copy for a finished head
                pb = (h % 2) * 64
                j = h // 2
                rec = recp.tile([1, LQ], f32, tag="rec")
                nc.vector.reciprocal(rec, ops[64:65, 0:LQ])
                rbc = recp.tile([64, LQ], f32, tag="rbc")
                nc.gpsimd.partition_broadcast(rbc, rec)
                nc.vector.tensor_mul(
                    out=outT[pb:pb + 64, j, 0:LQ],
                    in0=ops[0:64, 0:LQ], in1=rbc)

            # ---- phase 1: kT and V GEMMs, overlapped with head-0 attention
            # so the activation engine starts exp'ing ~45us earlier.
            NBLOCKS = [(0, 256), (256, 768), (768, 1280), (1280, 1792)]
            wpT_sb = main.tile([128, 4, C], f32r)
            psoA = tc.alloc_tile_pool(name="psoutA", bufs=1, space="PSUM")
            pspA = tc.alloc_tile_pool(name="psstA", bufs=2, space="PSUM")
            with (
                tc.tile_pool(name="wk", bufs=1) as wk,
                tc.tile_pool(name="xs", bufs=4) as xs,
                tc.tile_pool(name="psmm", bufs=2, space="PSUM") as psmm,
            ):
                psstA, psoutA = pspA, psoA
                wkT_sb = wk.tile([128, 8, 512], bf16)
                wvT_sb = wk.tile([128, 8, 512], bf16)
                wkTr = wkT.rearrange("(j p) m -> p j m", p=128)
                kauxr = kaux.ap()
                # DMA emission order tracks the serial-device schedule: the
                # head-0 prerequisites (kaux block 0, qT, its bias, ident)
                # are squeezed in right after the first x/w loads.
                nc.sync.dma_start(wkT_sb[:, :, 0:256], wkTr[:, :, 0:256])
                xblks = []
                bh0 = biasp.tile([128, WTOT], bf16, tag="bh")
                for bi, (n0, n1) in enumerate(NBLOCKS):
                    w = n1 - n0
                    xblk = xs.tile([128, 8, 512], bf16, tag="xblk")
                    nc.sync.dma_start(xblk[:, :, 0:w], xTr[:, :, n0:n1])
                    xblks.append(xblk)
                    if bi == 0:
                        nc.sync.dma_start(
                            wkT_sb[:, :, 256:512], wkTr[:, :, 256:512])
                    elif bi == 1:
                        nc.sync.dma_start(
                            kT_sb[64:66, :, 13 * 128:NPAD], kauxr)
                        nc.sync.dma_start(ident_sb, identb.ap())
                    elif bi == 2:
                        nc.sync.dma_start(bh0, biasw.ap()[0])
                        nc.sync.dma_start(qT_sb, qT.ap())
                    elif bi == 3:
                        nc.sync.dma_start(
                            wvT_sb, wvT.rearrange("(j p) m -> p j m", p=128))

                # ones aux row via the idle Pool engine, per block so the
                # first S tiles are ready early (row 65 is only read for
                # tile 13, whose aux cols arrive in the kaux DMA above)
                for (n0, n1) in NBLOCKS:
                    hi = min(n1, 13 * 128)
                    nc.gpsimd.memset(kT_sb[64:65, :, n0:hi], 1.0)

                def emit_kt(bi):
                    n0, n1 = NBLOCKS[bi]
                    w = n1 - n0
                    xblk = xblks[bi]
                    for mt in range(4):
                        ps = psmm.tile([128, 512], f32, tag="ps")
                        for kj in range(8):
                            nc.tensor.matmul(
                                ps[:, 0:w],
                                wkT_sb[:, kj, mt * 128:(mt + 1) * 128],
                                xblk[:, kj, 0:w],
                                start=(kj == 0), stop=(kj == 7),
                            )
                        nc.vector.tensor_copy(
                            kT_sb[0:64, 2 * mt, n0:n1], ps[0:64, 0:w])
                        nc.vector.tensor_copy(
                            kT_sb[0:64, 2 * mt + 1, n0:n1], ps[64:128, 0:w])

                def emit_v_tile(t):
                    bi = next(i for i, (a, b) in enumerate(NBLOCKS)
                              if a <= t * 128 < b)
                    n0, _ = NBLOCKS[bi]
                    xblk = xblks[bi]
                    rel = t * 128 - n0
                    ps = psmm.tile([128, 512], f32, tag="ps")
                    for kj in range(8):
                        nc.tensor.matmul(
                            ps,
                            xblk[:, kj, rel:rel + 128],
                            wvT_sb[:, kj, :],
                            start=(kj == 0), stop=(kj == 7),
                        )
                    nc.vector.tensor_copy(
                        vre[:, t, :, 0:64],
                        ps.rearrange("p (h e) -> p h e", e=64),
                    )

                def tiles_of(bi):
                    n0, n1 = NBLOCKS[bi]
                    return range(n0 // 128, n1 // 128)

                # prefetch every head's bias windows now; transfers slot
                # into DMA-pipe idle behind the x/w loads
                bhs = {0: bh0}
                for h in range(1, HPC - 2):
                    bhs[h] = biasp.tile(
                        [128, WTOT], bf16, tag="bh", name=f"bh{h}")
                    nc.sync.dma_start(bhs[h], biasw.ap()[h])

                ops0 = psoutA.tile([66, QPAD], f32, tag="ops")
                pts = {}
                for b in range(len(NBLOCKS)):
                    emit_kt(b)
                prev = None
                for t in range(NT):
                    emit_v_tile(t)
                    pts[t] = emit_sexp(0, t, psstA, bh0)
                    if prev is not None:
                        emit_pv(0, prev, pts.pop(prev), ops0)
                    prev = t
                emit_pv(0, prev, pts.pop(prev), ops0)
                emit_epilogue(0, ops0)

            # ---- phase 3: heads 1..7 on the same PSUM pools. S/exp for
            # a whole head is emitted before its PVs (deep pt pipeline), so
            # the single ops buffer and the activation engine stay busy
            # across head boundaries.
            pspA.release()
            pspC = tc.alloc_tile_pool(name="pspC", bufs=2, space="PSUM")
            for h in range(1, HPC):
                if h in (2, 3):  # late bias tables: reuse freed buffers
                    hh = h + 4
                    bhs[hh] = biasp.tile(
                        [128, WTOT], bf16, tag="bh", name=f"bh{hh}")
                    nc.sync.dma_start(bhs[hh], biasw.ap()[hh])
                bh = bhs[h]
                ops = psoA.tile([66, QPAD], f32, tag="ops")
                # single exps at both ends shorten the serial ACT chain's
                # lead-in (first PVs) and tail (last PVs) per head
                grans = [(0,), (1, 2), (3, 4), (5, 6), (7, 8), (9, 10),
                         (11, 12), (13,)]
                pt_of = {}
                for g in grans:
                    st2 = pspC.tile([128, 2 * QPAD], f32, tag="st2")
                    for i, t in enumerate(g):
                        emit_tile_at(h, t, st2, i * QPAD, bh)
                    if len(g) == 1:
                        pt = ptp.tile([128, QPAD], bf16, tag="pt")
                        nc.scalar.activation(
                            pt, st2[:, 0:QPAD],
                            mybir.ActivationFunctionType.Exp)
                        pt_of[g[0]] = (pt, 0)
                    else:
                        pt2 = ptp2.tile([128, 2 * QPAD], bf16, tag="pt2")
                        nc.scalar.activation(
                            pt2, st2, mybir.ActivationFunctionType.Exp)
                        for i, t in enumerate(g):
                            pt_of[t] = (pt2, i * QPAD)
                if h == HPC - 1:
                    # st2 tiles die with the exps; free their banks now so
                    # the projection pool can allocate during the last PVs
                    pspC.release()
                for t in range(NT):
                    emit_pv_at(h, t, pt_of[t][0], pt_of[t][1], ops)
                emit_epilogue(h, ops)

            nc.sync.dma_start(wpT_sb, wpT.rearrange("(j p) n -> p j n", p=128))

            # ---- phase 4: projection (outT already normalized per head)
            with tc.tile_pool(name="pspj", bufs=3, space="PSUM") as pspj:
                for mt in range(5):
                    m0 = mt * 128
                    mcols = 68 if mt == 4 else 128
                    mrows = 65 if mt == 4 else 128
                    for (c0, c1) in ((0, 512), (512, C)):
                        ps = pspj.tile([128, 512], f32, tag="pp")
                        for j in range(4):
                            nc.tensor.matmul(
                                ps[:mcols], outT[:, j, m0:m0 + mcols],
                                wpT_sb[:, j, c0:c1],
                                start=(j == 0), stop=(j == 3),
                            )
                        yt = yp.tile([128, 512], bf16, tag="yt")
                        nc.vector.tensor_copy(yt[:mrows], ps[:mrows])
                        nc.sync.dma_start(
                            y.ap()[m0:m0 + mrows, c0:c1], yt[:mrows])
            psoA.release()

    nc.finalize()
    return nc


_NC_CACHE = None


def _get_nc():
    global _NC_CACHE
    if _NC_CACHE is None:
        _NC_CACHE = _build_nc()
    return _NC_CACHE


def _host_prep(x, q_learned, pos_embed, Wk, Wv, Wp, rpe_W, rp_bucket):
    """Build the 8 per-core input maps."""
    x = np.asarray(x, dtype=np.float32)
    q_ = (np.asarray(q_learned, np.float32) + np.asarray(pos_embed, np.float32))[0]
    Wk = np.asarray(Wk, np.float32)
    Wv = np.asarray(Wv, np.float32)
    Wp = np.asarray(Wp, np.float32)
    rpe_W = np.asarray(rpe_W, np.float32)
    rp_bucket = np.asarray(rp_bucket)

    scale = HD ** -0.5

    # RPE table per head/query: tab[h, q, n] (nb buckets, cls bucket = 4)
    qh = q_.reshape(LQ, H, HD)
    tab = np.einsum('qhd,dn->hqn', qh, rpe_W)                      # (H, LQ, nb)
    rpe = np.take_along_axis(
        tab, np.broadcast_to(rp_bucket[None], (H, LQ, LQ)), axis=-1
    )                                                              # (H, q, j')

    # rank-1 aux rows (bias = ones*A + is_cls*Cc + LOCAL)
    qxA = np.zeros((H, QPAD), np.float32)
    qxA[:, 0] = tab[:, 0, 4]
    qxA[:, 1:LQ] = tab[:, 1:, 3]
    qxC = np.zeros((H, QPAD), np.float32)
    qxC[:, 1:LQ] = tab[:, 1:, 4] - tab[:, 1:, 3]

    # full bias in device key order (spatial-major, cls last), minus rank-1
    n_idx = np.arange(NPAD)
    jcol = np.where(n_idx == CLS_KEY, 0, 1 + n_idx % P_SP)         # (NPAD,)
    local = rpe[:, :, jcol].transpose(0, 2, 1)                     # (H, n, q)
    local[:, CLS_KEY + 1:, :] = 0.0
    kxA = (n_idx <= CLS_KEY).astype(np.float32)
    kxC = (n_idx == CLS_KEY).astype(np.float32)
    local -= kxA[None, :, None] * qxA[:, None, :LQ]
    local -= kxC[None, :, None] * qxC[:, None, :LQ]

    # slice the per-tile windows; everything outside must be zero
    biasw = np.zeros((H, 128, WTOT), np.float32)
    off = 0
    check = local.copy()
    for t, parts in enumerate(WINDOWS):
        for (q0, q1) in parts:
            w = q1 - q0
            biasw[:, :, off:off + w] = local[:, t * 128:(t + 1) * 128, q0:q1]
            check[:, t * 128:(t + 1) * 128, q0:q1] = 0.0
            off += w
    assert np.abs(check).max() < 1e-6, "RPE residual outside windows"
    biasw = biasw.astype(ml_dtypes.bfloat16)

    # qT slots: rows 0:64 scaled head queries, 64/65 aux rows
    qTs = np.zeros((2, 128, HPC, QPAD), np.float32)
    qsc = (q_.T * scale).reshape(H, HD, LQ)                        # (H, 64, LQ)
    for g in range(2):
        for hl in range(HPC):
            h = g * HPC + hl
            qTs[g, 0:64, hl, :LQ] = qsc[h]
            qTs[g, 64, hl] = qxA[h]
            qTs[g, 65, hl] = qxC[h]
    qTs = qTs.astype(ml_dtypes.bfloat16)

    identb = np.eye(128, dtype=ml_dtypes.bfloat16)

    kaux = np.zeros((2, HPC, 128), np.float32)
    kaux[0] = 1.0                  # ones row (pad keys harmless: V is zero)
    kaux[1, :, CLS_KEY - 13 * 128] = 1.0
    kaux = kaux.astype(ml_dtypes.bfloat16)

    per_group = []
    for g in range(2):
        sl = slice(g * 512, (g + 1) * 512)
        per_group.append({
            "wkT": np.ascontiguousarray(Wk[sl, :].T).astype(ml_dtypes.bfloat16),
            "wvT": np.ascontiguousarray(Wv[sl, :].T).astype(ml_dtypes.bfloat16),
            "wpT": np.ascontiguousarray(Wp[:, sl].T),
            "qT": np.ascontiguousarray(qTs[g]),
            "identb": identb,
            "kaux": kaux,
            "biasw": np.ascontiguousarray(biasw[g * HPC:(g + 1) * HPC]),
        })

    in_maps = []
    for b in range(B):
        xTb = np.zeros((C, NPAD), np.float32)
        xTb[:, 0:CLS_KEY] = x[b, 1:].T        # spatial keys, modality-major
        xTb[:, CLS_KEY] = x[b, 0]             # cls key last
        xTb = np.ascontiguousarray(xTb.reshape(8, 128, NPAD)).astype(
            ml_dtypes.bfloat16)
        for g in range(2):
            m = dict(per_group[g])
            m["xT"] = xTb
            in_maps.append(m)
    return in_maps


def kernel(x, q_learned, pos_embed, Wk, Wv, Wp, bp, rpe_W, rp_bucket):
    in_maps = _host_prep(x, q_learned, pos_embed, Wk, Wv, Wp, rpe_W, rp_bucket)
    nc = _get_nc()

    last_err = None
    for _attempt in range(3):
        try:
            res = run_bass_kernel_spmd(nc, in_maps, core_ids=list(range(NCORES)))
            break
        except Exception as e:  # wedged-device recovery: retry
            last_err = e
    else:
        raise last_err

    bp = np.asarray(bp, np.float32)
    out = np.empty((B, LQ, C), np.float32)
    for b in range(B):
        out[b] = (res.results[2 * b]["y"].astype(np.float32)
                  + res.results[2 * b + 1]["y"].astype(np.float32) + bp)
    return out
